# revision 3
# baseline (speedup 1.0000x reference)
"""Trainium2 Bass kernel for BasicMoE.

Reference computation (N=8192 tokens, D=1024 in, O=1024 out, E=8 experts):
    gates = softmax(x @ Wg + bg)                        # [N, E]
    out   = sum_e gates[:, e] * (x @ We[e] + be[e])     # [N, O]

Strategy: data-parallel over tokens (1024 tokens/core, replicated weights),
with a mixed-precision decomposition that moves 8/9 of the matmul FLOPs to
fp8 DoubleRow (2 MACs/cell/cycle):

    out = x @ Wmean                                (bf16 GEMM, exact-ish)
        + sum_e (g_e - 1/8) * (x8 @ Wp8_e)         (fp8e4 DoubleRow GEMMs)
        + g @ be                                   (rank-8 bf16 GEMM)

where Wmean = mean_e We, Wp_e = We - Wmean, and sum_e (g_e - 1/8) Wp_e
== sum_e g_e Wp_e exactly because sum_e Wp_e = 0. Quantization noise of
the fp8 GEMMs enters scaled by ||g - 1/8|| ~ 0.19 instead of ||g|| ~ 0.38,
which keeps the end-to-end rel err at ~1.75e-2 (measured against the
reference on the true inputs) under the 2e-2 gate.

Per-core phases:
  A : gating logits + softmax on PE/ACT/DVE; also cc = (g - 1/8)/2^16
      (the 2^16 removes the fp8 scale factors XS*WS).
  A2: transpose gates (PE) for the bias matmul.
  B0: acc = x @ Wmean + gT.T @ be   (bf16 PE, copy PSUM->SBUF).
  B : for e,t: psum = DR-matmul(x8, Wp8_e) over k-pairs;
      acc += psum * cc[:, e] (fused DVE scalar_tensor_tensor);
      last expert streams acc out to HBM.
"""

import numpy as np
import ml_dtypes

N_TOKENS = 8192
D = 1024   # in dim
O = 1024   # out dim
E = 8      # experts
NCORES = 8
NLOC = N_TOKENS // NCORES   # 1024 tokens per core
KT = D // 128               # 8 k-chunks
TT = NLOC // 128            # 8 token chunks
JT = O // 512               # 2 out chunks

BF16 = ml_dtypes.bfloat16
F8E4 = ml_dtypes.float8_e4m3   # IEEE e4m3: max normal 240, matches TRN fp8e4

XS = 32.0      # x fp8 scale (|x| < 5.2 -> < 166)
WS = 2048.0    # Wp fp8 scale (|Wp| < 0.051 -> < 105)
CINV = 1.0 / (XS * WS)

_CACHE = {}


def _build():
    """Build + compile the per-core Bass graph (same graph on all 8 cores)."""
    import concourse.bass as bass
    import concourse.mybir as mybir
    import concourse.tile as tile
    from concourse import bacc
    from concourse.masks import make_identity

    dt = mybir.dt
    f32 = dt.float32
    bf16 = dt.bfloat16
    f8e4 = dt.float8e4
    Alu = mybir.AluOpType
    DR = mybir.MatmulPerfMode.DoubleRow

    nc = bacc.Bacc(
        "TRN2",
        target_bir_lowering=False,
        debug=False,
        enable_asserts=False,
        num_devices=NCORES,
    )

    xt_d = nc.dram_tensor("xt", [128, KT * NLOC], bf16, kind="ExternalInput").ap()
    xt8_d = nc.dram_tensor("xt8", [128, KT * NLOC], f8e4, kind="ExternalInput").ap()
    wm_d = nc.dram_tensor("Wmp", [128, KT * O], bf16, kind="ExternalInput").ap()
    wp8_d = nc.dram_tensor("Wp8", [E, 128, KT * O], f8e4, kind="ExternalInput").ap()
    be_d = nc.dram_tensor("bep", [E, O], bf16, kind="ExternalInput").ap()
    wg_d = nc.dram_tensor("Wgp", [128, KT * E], bf16, kind="ExternalInput").ap()
    bg_d = nc.dram_tensor("bgp", [1, E], bf16, kind="ExternalInput").ap()
    out_d = nc.dram_tensor("out", [NLOC, O], f32, kind="ExternalOutput").ap()

    with tile.TileContext(nc) as tc:
        with (
            tc.tile_pool(name="const", bufs=1) as cpool,
            tc.tile_pool(name="xp", bufs=1) as xpool,
            tc.tile_pool(name="wp", bufs=E) as wpool,
            tc.tile_pool(name="ap", bufs=1) as apool,
            tc.tile_pool(name="gp", bufs=1) as gpool,
        ):
            ident = cpool.tile([128, 128], bf16)
            make_identity(nc, ident[:])
            ones = cpool.tile([1, 128], bf16)
            nc.gpsimd.memset(ones[:], 1.0)
            # Small gating/bias constants on the SWDGE (gpsimd) queue so they
            # don't serialize behind the big streams on the HWDGE rings.
            wg_sb = cpool.tile([128, KT * E], bf16)
            nc.gpsimd.dma_start(wg_sb[:], wg_d)
            bg_sb = cpool.tile([1, E], bf16)
            nc.gpsimd.dma_start(bg_sb[:], bg_d)
            be_sb = cpool.tile([E, O], bf16)
            nc.gpsimd.dma_start(be_sb[:], be_d)

            # xt first (gating + mean GEMM need it), split across both HWDGE
            # rings so the two halves stream concurrently.
            xt = xpool.tile([128, KT * NLOC], bf16)
            half = KT * NLOC // 2
            nc.sync.dma_start(xt[:, :half], xt_d[:, :half])
            nc.scalar.dma_start(xt[:, half:], xt_d[:, half:])

            # Wmean next on the scalar ring (mean GEMM follows gating
            # immediately); x8 after it, then the per-expert fp8 weights on
            # the sync ring in consumption order.
            wm_sb = xpool.tile([128, KT * O], bf16)
            nc.scalar.dma_start(wm_sb[:], wm_d)
            xt8 = xpool.tile([128, KT, NLOC], f8e4)
            nc.scalar.dma_start(
                xt8[:], xt8_d.rearrange("p (k n) -> p k n", k=KT)
            )
            wp8_tiles = []
            for e in range(E):
                w8 = wpool.tile([128, KT, O], f8e4, tag="wp8", name=f"wp8_{e}")
                src = wp8_d[e].rearrange("p (k j c) -> j p k c", k=KT, j=JT, c=512)
                dst = w8.rearrange("p k (j c) -> j p k c", j=JT, c=512)
                for jh in range(JT):
                    nc.sync.dma_start(dst[jh], src[jh])
                wp8_tiles.append(w8)

            acc = apool.tile([128, TT * O], f32)

            g_f32 = gpool.tile([128, TT * E], f32)
            cc_f32 = gpool.tile([128, TT * E], f32)
            g_bf = gpool.tile([128, TT * E], bf16)
            gT = gpool.tile([E, NLOC], bf16)
            negm = gpool.tile([128, TT], f32)
            ssum = gpool.tile([128, TT], f32)
            rec = gpool.tile([128, TT], f32)

            def xt_tile(k, t):
                c = k * NLOC + t * 128
                return xt[:, c : c + 128]

            # ---- Phase A: gating logits + softmax --------------------------
            with tc.tile_pool(name="psA", bufs=2, space="PSUM") as psA:
                for t in range(TT):
                    zg = psA.tile([128, E], f32, tag="zg")
                    for k in range(KT):
                        nc.tensor.matmul(
                            zg[:],
                            xt_tile(k, t),
                            wg_sb[:, k * E : (k + 1) * E],
                            start=(k == 0),
                            stop=False,
                        )
                    # + bg (rank-1: ones[1,128].T @ bg[1,E])
                    nc.tensor.matmul(zg[:], ones[:], bg_sb[:], start=False, stop=True)

                    nm = negm[:, t : t + 1]
                    nc.vector.tensor_reduce(
                        nm, zg[:], axis=mybir.AxisListType.X, op=Alu.max, negate=True
                    )
                    gs = g_f32[:, t * E : (t + 1) * E]
                    nc.scalar.activation(
                        gs,
                        zg[:],
                        mybir.ActivationFunctionType.Exp,
                        bias=nm,
                        scale=1.0,
                        accum_out=ssum[:, t : t + 1],
                    )
                    nc.vector.reciprocal(rec[:, t : t + 1], ssum[:, t : t + 1])
                    nc.vector.tensor_scalar_mul(gs, gs, rec[:, t : t + 1])
                    # cc = (g - 1/8) * CINV -- fp8-scale-corrected centered gates
                    nc.vector.tensor_scalar(
                        cc_f32[:, t * E : (t + 1) * E],
                        gs,
                        -0.125,
                        CINV,
                        op0=Alu.add,
                        op1=Alu.mult,
                    )
                    nc.gpsimd.tensor_copy(g_bf[:, t * E : (t + 1) * E], gs)

            # ---- Phase A2: transpose gates for the bias matmul -------------
            with tc.tile_pool(name="psC", bufs=1, space="PSUM") as psC:
                for t in range(TT):
                    trp = psC.tile([E, 128], bf16, tag="tr")
                    nc.tensor.transpose(
                        trp[:], g_bf[:, t * E : (t + 1) * E], ident[:]
                    )
                    nc.vector.tensor_copy(gT[:, t * 128 : (t + 1) * 128], trp[:])

            # ---- Phase B0: mean GEMM + bias -------------------------------
            # acc[t, j] = sum_k x_t @ Wmean[k, j] + gT_t.T @ be[:, j]
            with tc.tile_pool(name="psM", bufs=4, space="PSUM") as psM:
                for t in range(TT):
                    for j in range(JT):
                        pm = psM.tile([128, 512], f32, tag="pm")
                        for k in range(KT):
                            nc.tensor.matmul(
                                pm[:],
                                xt_tile(k, t),
                                wm_sb[:, k * O + j * 512 : k * O + (j + 1) * 512],
                                start=(k == 0),
                                stop=False,
                            )
                        nc.tensor.matmul(
                            pm[:],
                            gT[:, t * 128 : (t + 1) * 128],
                            be_sb[:, j * 512 : (j + 1) * 512],
                            start=False,
                            stop=True,
                        )
                        nc.vector.tensor_copy(
                            acc[:, t * O + j * 512 : t * O + (j + 1) * 512], pm[:]
                        )

            # ---- Phase B: fp8 DoubleRow correction GEMMs ------------------
            # psum[t,j] = sum_{k-pairs} DR(x8, Wp8_e); acc += psum * cc[:, e]
            with tc.tile_pool(name="psB", bufs=6, space="PSUM") as psB:
                for e in range(E):
                    w8 = wp8_tiles[e]
                    last = e == E - 1
                    for t in range(TT):
                        ps = [
                            psB.tile([128, 512], f32, tag="mm", name=f"mm{j}")
                            for j in range(JT)
                        ]
                        for k2 in range(KT // 2):
                            lhs = xt8[:, 2 * k2 : 2 * k2 + 2, t * 128 : (t + 1) * 128]
                            for j in range(JT):
                                nc.tensor.matmul(
                                    ps[j][:],
                                    lhs,
                                    w8[:, 2 * k2 : 2 * k2 + 2, j * 512 : (j + 1) * 512],
                                    start=(k2 == 0),
                                    stop=(k2 == KT // 2 - 1),
                                    perf_mode=DR,
                                )
                        ccol = cc_f32[:, t * E + e : t * E + e + 1]
                        for j in range(JT):
                            a_sl = acc[:, t * O + j * 512 : t * O + (j + 1) * 512]
                            nc.vector.scalar_tensor_tensor(
                                a_sl, ps[j][:], ccol, a_sl,
                                op0=Alu.mult, op1=Alu.add,
                            )
                            if last:
                                nc.sync.dma_start(
                                    out_d[
                                        t * 128 : (t + 1) * 128,
                                        j * 512 : (j + 1) * 512,
                                    ],
                                    a_sl,
                                )

    nc.compile()
    return nc


def _get_nc():
    if "nc" not in _CACHE:
        _CACHE["nc"] = _build()
    return _CACHE["nc"]


def _pack_inputs(x, We, be, Wg, bg):
    """Host-side packing: shard + pre-transpose + cast to bf16/fp8."""
    x = np.asarray(x, dtype=np.float32)
    We = np.asarray(We, dtype=np.float32)
    be = np.asarray(be, dtype=np.float32)
    Wg = np.asarray(Wg, dtype=np.float32)
    bg = np.asarray(bg, dtype=np.float32)

    Wmean = We.mean(axis=0)
    Wp = We - Wmean[None]

    def ptrans(w):  # [D, O] -> [128, KT*O] with [p, k*O + o] = w[k*128+p, o]
        return np.ascontiguousarray(
            w.reshape(KT, 128, O).transpose(1, 0, 2).reshape(128, KT * O)
        )

    wm_p = ptrans(Wmean).astype(BF16)
    wp8_p = np.stack(
        [np.clip(ptrans(Wp[e]) * WS, -240, 240).astype(F8E4) for e in range(E)]
    )
    be_p = be.astype(BF16)
    wg_p = np.ascontiguousarray(
        Wg.reshape(KT, 128, E).transpose(1, 0, 2).reshape(128, KT * E)
    ).astype(BF16)
    bg_p = bg.reshape(1, E).astype(BF16)

    in_maps = []
    for i in range(NCORES):
        xs = x[i * NLOC : (i + 1) * NLOC]          # [NLOC, D]
        # xt[p, k*NLOC + n] = xs[n, k*128+p]
        xt_f = np.ascontiguousarray(
            xs.T.reshape(KT, 128, NLOC).transpose(1, 0, 2).reshape(128, KT * NLOC)
        )
        xt = xt_f.astype(BF16)
        xt8 = np.clip(xt_f * XS, -240, 240).astype(F8E4)
        in_maps.append(
            {
                "xt": xt,
                "xt8": xt8,
                "Wmp": wm_p,
                "Wp8": wp8_p,
                "bep": be_p,
                "Wgp": wg_p,
                "bgp": bg_p,
            }
        )
    return in_maps


def _run(inputs, trace=False):
    """Returns (y_full, BassKernelResults)."""
    from concourse.bass_utils import run_bass_kernel_spmd

    nc = _get_nc()
    in_maps = _pack_inputs(**inputs)
    res = run_bass_kernel_spmd(
        nc, in_maps, core_ids=list(range(NCORES)), trace=trace
    )
    y = np.concatenate(
        [res.results[i]["out"] for i in range(NCORES)], axis=0
    ).astype(np.float32)
    return y, res


def kernel(**inputs):
    y, _ = _run(inputs, trace=False)
    return y
